# revision 1
# baseline (speedup 1.0000x reference)
"""GQA attention (B=2, S=2048, D=2048, 32 q heads / 8 kv heads, RoPE, causal)
sharded over 8 trn2 NeuronCores: tensor-parallel over heads (4 q heads + 1 kv
head per core), AllToAll to redistribute attention output by sequence slice,
each core computes its 512-row slice of the output projection.

Matmuls run as float32r (fp32 storage, relaxed-precision full-rate PE mode).
The PV matmul uses lhsT = [V | ones] so the softmax denominator accumulates in
PSUM rows 64:128 broadcast across partitions; causal masking is an additive
mask matmul into the scores accumulation (exp then yields exact zeros).

Self-contained: hardcodes shapes; builds one SPMD Bass/Tile program and runs it
via run_bass_kernel_spmd on cores 0-7.
"""

import os
import sys

import numpy as np

for _p in ("/opt/trn_rl_repo", "/root/.axon_site/_ro/trn_rl_repo"):
    if os.path.isdir(_p) and _p not in sys.path:
        sys.path.insert(0, _p)

B = 2
S = 2048
D = 2048
HD = 64          # head dim
HQ = 4           # q heads per core
NCORES = 8
ROPE_THETA = 10000.0
P = 128

_CACHE: dict = {}


def _build_program(phases=("xqkv", "attn", "cc", "proj"), reps=1):
    import concourse.bacc as bacc
    import concourse.tile as tile
    from concourse import mybir
    from concourse.masks import make_identity
    import concourse.hw_specs as hw_specs

    # Pin every activation we use (Exp/Ln/Copy) to the one table set that has
    # them all, so the table-load pass emits a single LoadActFuncSet instead of
    # thrashing between sets on every Ln <-> Exp transition.
    if not hasattr(bacc, "_act_tables_pinned"):
        _orig_gat = hw_specs.get_activation_tables
        _AF = mybir.ActivationFunctionType
        _ours = {_AF.Exp, _AF.Ln, _AF.Copy, _AF.Identity}

        def _gat(arch):
            tabs = _orig_gat(arch)
            for name in tabs:
                if name != "natural_log_exp_and_others":
                    tabs[name] = tabs[name] - _ours
            return tabs

        bacc.get_activation_tables = _gat
        bacc._act_tables_pinned = True

    dt = mybir.dt.float32
    Rdt = mybir.dt.float32r
    AF = mybir.ActivationFunctionType
    OP = mybir.AluOpType

    nc = bacc.Bacc("TRN2", target_bir_lowering=False, debug=False, num_devices=NCORES)

    x = nc.dram_tensor("x", [B, S, D], Rdt, kind="ExternalInput")
    wqT = nc.dram_tensor("wqT", [D, 256], Rdt, kind="ExternalInput")
    wk2T = nc.dram_tensor("wk2T", [D, 128], Rdt, kind="ExternalInput")
    wvT = nc.dram_tensor("wvT", [D, 64], Rdt, kind="ExternalInput")
    woT = nc.dram_tensor("woT", [D, D], Rdt, kind="ExternalInput")
    ctab = nc.dram_tensor("ctab", [P, S], dt, kind="ExternalInput")
    stab = nc.dram_tensor("stab", [P, S], dt, kind="ExternalInput")
    pswT = nc.dram_tensor("pswT", [P, P], Rdt, kind="ExternalInput")
    masks4 = nc.dram_tensor("masks4", [4 * P, 512], Rdt, kind="ExternalInput")
    out = nc.dram_tensor("out", [512, D], dt, kind="ExternalOutput")
    a2a_in = [[nc.dram_tensor(f"a2a_in{i}_{q}", [2048, 128], Rdt) for q in range(2)] for i in range(B)]
    a2a_out = [[nc.dram_tensor(f"a2a_out{i}_{q}", [2048, 128], Rdt) for q in range(2)] for i in range(B)]

    with tile.TileContext(nc) as tc:
        with tc.tile_pool(name="singles", bufs=1) as singles:
            ident = singles.tile([P, P], dt)
            make_identity(nc, ident)
            identR = singles.tile([P, P], Rdt)
            nc.vector.tensor_copy(identR, ident)
            ones_f = singles.tile([P, HD], dt)
            nc.vector.memset(ones_f, 1.0)
            wq_sb = singles.tile([P, 16, 256], Rdt)
            nc.sync.dma_start(wq_sb, wqT.ap().rearrange("(ko p) m -> p ko m", p=P))
            wk_sb = singles.tile([P, 16, 128], Rdt)
            nc.sync.dma_start(wk_sb, wk2T.ap().rearrange("(ko p) m -> p ko m", p=P))
            wv_sb = singles.tile([P, 16, 64], Rdt)
            nc.sync.dma_start(wv_sb, wvT.ap().rearrange("(ko p) m -> p ko m", p=P))
            psw_sb = singles.tile([P, P], Rdt)
            nc.sync.dma_start(psw_sb, pswT.ap())
            c_sb = singles.tile([P, S], dt)
            nc.sync.dma_start(c_sb, ctab.ap())
            s_sb = singles.tile([P, S], dt)
            nc.sync.dma_start(s_sb, stab.ap())
            msk_sb = singles.tile([P, 4, 512], Rdt)
            nc.sync.dma_start(msk_sb, masks4.ap().rearrange("(a p) q -> p a q", p=P))

            for rep in range(reps):
                for b in range(B):
                    with tc.tile_pool(name=f"qkv{rep}_{b}", bufs=1) as qkv:
                        qT = qkv.tile([P, 2, S], Rdt)       # 2 head-pairs [A(0:64)|B(64:128)]
                        k2T = qkv.tile([P, S], Rdt)         # kv head duplicated in both halves
                        vsb = qkv.tile([P, 16, P], Rdt)     # [s-part, kc, (V 0:64 | ones 64:128)]
                        nc.vector.tensor_copy(
                            vsb[:, :, HD:P], ones_f[:, None, :].to_broadcast([P, 16, HD]))

                        # ---- x^T + QKV projections + RoPE, per 512-col s-block ----
                        with tc.tile_pool(name=f"xt{rep}_{b}", bufs=2) as xtp, \
                             tc.tile_pool(name=f"xnat{rep}_{b}", bufs=3) as xnp, \
                             tc.tile_pool(name=f"tmp{rep}_{b}", bufs=4) as tmp, \
                             tc.tile_pool(name=f"pst{rep}_{b}", bufs=2, space="PSUM") as pst, \
                             tc.tile_pool(name=f"psq{rep}_{b}", bufs=2, space="PSUM") as psq, \
                             tc.tile_pool(name=f"psw{rep}_{b}", bufs=2, space="PSUM") as psw:
                            for n4 in range(4):
                                xTblk = xtp.tile([P, 16, 512], Rdt, tag="xT")
                                for sc4 in range(4):
                                    sc = n4 * 4 + sc4
                                    xa = xnp.tile([P, D], Rdt, tag="xa")
                                    nc.sync.dma_start(xa, x.ap()[b, sc * P:(sc + 1) * P, :])
                                    for dg in range(4):
                                        pt = pst.tile([P, 512], Rdt, tag="pt", bufs=3)
                                        for j in range(4):
                                            dc = dg * 4 + j
                                            nc.tensor.transpose(
                                                pt[:, j * P:(j + 1) * P],
                                                xa[:, dc * P:(dc + 1) * P],
                                                identR,
                                            )
                                        dest = xTblk[:, dg * 4:dg * 4 + 4, sc4 * P:(sc4 + 1) * P]
                                        nc.vector.tensor_copy(
                                            dest, pt[:, :].rearrange("p (a c) -> p a c", a=4)
                                        )
                                # m=0,1: Q pairs; m=2: duplicated kv K; all get RoPE
                                for m in range(3):
                                    ps = psq.tile([P, 512], dt, tag="ps")
                                    for k in range(16):
                                        lhsT = wq_sb[:, k, m * P:(m + 1) * P] if m < 2 else wk_sb[:, k, :]
                                        nc.tensor.matmul(
                                            ps, lhsT, xTblk[:, k, :],
                                            start=(k == 0), stop=(k == 15),
                                        )
                                    raw = tmp.tile([P, 512], Rdt, tag="raw")
                                    nc.scalar.copy(out=raw, in_=ps)
                                    sw = psw.tile([P, 512], dt, tag="sw", bufs=1)
                                    nc.tensor.matmul(sw, psw_sb, raw, start=True, stop=True)
                                    u = tmp.tile([P, 512], dt, tag="u")
                                    nc.vector.tensor_tensor(
                                        u, sw, s_sb[:, n4 * 512:(n4 + 1) * 512], OP.mult)
                                    t2 = tmp.tile([P, 512], dt, tag="t2")
                                    nc.vector.tensor_tensor(
                                        t2, raw, c_sb[:, n4 * 512:(n4 + 1) * 512], OP.mult)
                                    dest = qT[:, m, n4 * 512:(n4 + 1) * 512] if m < 2 \
                                        else k2T[:, n4 * 512:(n4 + 1) * 512]
                                    nc.vector.tensor_tensor(dest, u, t2, OP.add)
                                # V^T for this s-block (no RoPE), then transpose into vsb
                                psv = psq.tile([HD, 512], dt, tag="psv", bufs=1)
                                for k in range(16):
                                    nc.tensor.matmul(
                                        psv, wv_sb[:, k, :], xTblk[:, k, :],
                                        start=(k == 0), stop=(k == 15),
                                    )
                                vT = tmp.tile([HD, 512], Rdt, tag="vT")
                                nc.scalar.copy(out=vT, in_=psv)
                                pvt = pst.tile([P, 256], Rdt, tag="pvt", bufs=1)
                                for j in range(4):
                                    nc.tensor.transpose(
                                        pvt[:, j * HD:(j + 1) * HD],
                                        vT[:, j * P:(j + 1) * P],
                                        identR[:HD, :HD],
                                    )
                                nc.vector.tensor_copy(
                                    vsb[:, n4 * 4:n4 * 4 + 4, 0:HD],
                                    pvt[:, :].rearrange("p (a c) -> p a c", a=4),
                                )

                        # ---- attention, orientation B (scores^T [k, q]) ----
                        # qh outer so each (b, qh) column range finishes early and
                        # its AllToAll overlaps the remaining attention compute.
                        if "attn" in phases:
                            with tc.tile_pool(name=f"at{rep}_{b}", bufs=4) as atp, \
                                 tc.tile_pool(name=f"ps_s{rep}_{b}", bufs=2, space="PSUM") as pss, \
                                 tc.tile_pool(name=f"ps_pv{rep}_{b}", bufs=2, space="PSUM") as pspv:
                              for qh in range(2):
                                for p in range(2):  # head pair
                                    qcs = [2 * qh, 2 * qh + 1]
                                    pvd = {(qc, h): pspv.tile([P, 512], dt, name="pvd", tag=f"pv{h}")
                                           for qc in qcs for h in range(2)}
                                    for kc in range(4 * qcs[-1] + 4):
                                        for qc in qcs:
                                            if kc >= 4 * (qc + 1):
                                                continue
                                            diag = kc >= 4 * qc
                                            # columns [0, skip) of this unit are fully
                                            # masked (q < k for every partition): skip
                                            # them in scores/exp/PV. Cap skip at 256 so
                                            # f32r matmuls keep N>=256 full rate.
                                            skip = min(P * (kc - 4 * qc), 256) if diag else 0
                                            w = 512 - skip
                                            sq = pss.tile([P, 1024], dt, name="sq", tag="sq")
                                            for h in range(2):
                                                nc.tensor.matmul(
                                                    sq[:, h * 512 + skip:(h + 1) * 512],
                                                    k2T[64 * h:64 * (h + 1), kc * P:(kc + 1) * P],
                                                    qT[64 * h:64 * (h + 1), p, qc * 512 + skip:(qc + 1) * 512],
                                                    start=True, stop=not diag,
                                                    tile_position=(64 * h, 0),
                                                    skip_group_check=True,
                                                )
                                                if diag:  # additive causal mask (-1e9 on masked)
                                                    nc.tensor.matmul(
                                                        sq[:, h * 512 + skip:(h + 1) * 512],
                                                        identR, msk_sb[:, kc - 4 * qc, skip:],
                                                        start=False, stop=True,
                                                        skip_group_check=True,
                                                    )
                                            pq = atp.tile([P, 1024], Rdt, name="pq", tag="pq")
                                            if skip:
                                                nc.scalar.activation(
                                                    pq[:, skip:512], sq[:, skip:512], AF.Exp)
                                                nc.scalar.activation(
                                                    pq[:, 512 + skip:1024], sq[:, 512 + skip:1024], AF.Exp)
                                            else:
                                                nc.scalar.activation(pq, sq, AF.Exp)
                                            first = kc == 0
                                            last = kc == 4 * (qc + 1) - 1
                                            # fused PV + denominator: lhsT = [V | ones]
                                            for h in range(2):
                                                nc.tensor.matmul(
                                                    pvd[(qc, h)][:, skip:512], vsb[:, kc, :],
                                                    pq[:, h * 512 + skip:(h + 1) * 512],
                                                    start=first, stop=last,
                                                    skip_group_check=True)
                                    for qc in qcs:
                                        for h in range(2):
                                            t_ = pvd[(qc, h)]
                                            ln = atp.tile([HD, 512], dt, tag="ln")
                                            nc.scalar.activation(ln, t_[HD:P, :], AF.Ln)
                                            rc = atp.tile([HD, 512], dt, tag="rc")
                                            nc.scalar.activation(rc, ln, AF.Exp, scale=-1.0)
                                            at = atp.tile([HD, 512], Rdt, tag="at")
                                            nc.vector.tensor_tensor(at, t_[0:HD, :], rc, OP.mult)
                                            for jj in range(4):
                                                j = 4 * (qc - 2 * qh) + jj
                                                row = 256 * j + P * p + HD * h
                                                nc.sync.dma_start(
                                                    a2a_in[b][qh].ap()[row:row + HD, :],
                                                    at[:, P * jj:P * (jj + 1)])
                                if "cc" in phases and p == 1:
                                    nc.gpsimd.collective_compute(
                                        "AllToAll",
                                        mybir.AluOpType.bypass,
                                        replica_groups=[list(range(NCORES))],
                                        ins=[a2a_in[b][qh].ap().opt()],
                                        outs=[a2a_out[b][qh].ap().opt()],
                                    )


                # ---- output projection: this core's 256-col s-slice of each b ----
                if "proj" in phases:
                    with tc.tile_pool(name=f"gp{rep}", bufs=1) as gp, \
                         tc.tile_pool(name=f"wop{rep}", bufs=2) as wop, \
                         tc.tile_pool(name=f"otmp{rep}", bufs=3) as otmp, \
                         tc.tile_pool(name=f"pso{rep}", bufs=2, space="PSUM") as pso:
                        # wo loads can start during b=1 attention (emitted first)
                        wos = []
                        for n in range(4):
                            wo_sb = wop.tile([P, 16, 512], Rdt, tag="wo", bufs=3, name=f"wo{n}")
                            nc.sync.dma_start(
                                wo_sb, woT.ap()[:, n * 512:(n + 1) * 512].rearrange("(ko p) f -> p ko f", p=P))
                            wos.append(wo_sb)
                        g_sbs = {}
                        for bb in range(B):
                            for q2 in range(2):
                                g_sb = gp.tile([P, 16, P], Rdt, name=f"g{bb}{q2}")
                                nc.sync.dma_start(
                                    g_sb, a2a_out[bb][q2].ap().rearrange("(ko p) q -> p ko q", p=P))
                                g_sbs[(bb, q2)] = g_sb
                        for n in range(4):
                            for bb in range(B):
                                for q2 in range(2):
                                    po = pso.tile([P, 512], dt, tag="po")
                                    for k in range(16):
                                        nc.tensor.matmul(
                                            po, g_sbs[(bb, q2)][:, k, :], wos[n][:, k, :],
                                            start=(k == 0), stop=(k == 15))
                                    ob = otmp.tile([P, 512], dt, tag="ob")
                                    nc.vector.tensor_copy(ob, po)
                                    nc.sync.dma_start(
                                        out.ap()[256 * bb + P * q2:256 * bb + P * (q2 + 1),
                                                 n * 512:(n + 1) * 512], ob)

    nc.compile()
    return nc


def _host_prep(Wq, Wk, Wv, Wo):
    """Per-core weight slices (head-dim permuted, transposed) + rope tables."""
    perm = np.concatenate([np.arange(0, HD, 2), np.arange(1, HD, 2)])
    per_core = []
    for r in range(NCORES):
        wq = Wq[256 * r:256 * (r + 1)].reshape(HQ, HD, D)[:, perm].reshape(256, D)
        wqT = np.ascontiguousarray(wq.T) * np.float32(0.125)
        wk = Wk[HD * r:HD * (r + 1)][perm]
        wk2T = np.ascontiguousarray(np.concatenate([wk, wk], 0).T)
        wvT = np.ascontiguousarray(Wv[HD * r:HD * (r + 1)].T)
        per_core.append((wqT, wk2T, wvT))
    woT = np.ascontiguousarray(Wo.T)

    half = HD // 2
    inv = 1.0 / (ROPE_THETA ** (np.arange(half, dtype=np.float64) * 2.0 / HD))
    ang = np.arange(S, dtype=np.float64)[None, :] * inv[:, None]  # [32, S]
    ctab = np.ascontiguousarray(np.tile(np.cos(ang), (4, 1)).astype(np.float32))
    stab = np.ascontiguousarray(np.tile(np.sin(ang), (4, 1)).astype(np.float32))

    I32 = np.eye(32, dtype=np.float32)
    z = np.zeros((32, 32), np.float32)
    blk = np.block([[z, -I32], [I32, z]])
    pswT = np.ascontiguousarray(np.kron(np.eye(2, dtype=np.float32), blk).T)
    kk = np.arange(P)[:, None]
    qq = np.arange(512)[None, :]
    masks4 = np.concatenate(
        [np.where(qq >= 128 * j + kk, 0.0, -1e9).astype(np.float32) for j in range(4)], axis=0)
    return per_core, woT, ctab, stab, pswT, masks4


def _get_nc():
    if "nc" not in _CACHE:
        _CACHE["nc"] = _build_program()
    return _CACHE["nc"]


def make_in_maps(x, Wq, Wk, Wv, Wo):
    x = np.ascontiguousarray(np.asarray(x, np.float32))
    Wq = np.asarray(Wq, np.float32)
    Wk = np.asarray(Wk, np.float32)
    Wv = np.asarray(Wv, np.float32)
    Wo = np.asarray(Wo, np.float32)
    per_core, woT, ctab, stab, pswT, masks4 = _host_prep(Wq, Wk, Wv, Wo)
    in_maps = []
    for r in range(NCORES):
        wqT, wk2T, wvT = per_core[r]
        in_maps.append({
            "x": x, "wqT": wqT, "wk2T": wk2T, "wvT": wvT, "woT": woT,
            "ctab": ctab, "stab": stab, "pswT": pswT, "masks4": masks4,
        })
    return in_maps


def unshard(per_core_outs):
    """per_core_outs[r]: [512, D] — blocks of 128 rows = (b, qh) slices."""
    full = np.empty((B * S, D), np.float32)
    for r in range(NCORES):
        o = per_core_outs[r]
        for b in range(B):
            for qh in range(2):
                full[S * b + 1024 * qh + P * r:S * b + 1024 * qh + P * (r + 1)] = \
                    o[256 * b + P * qh:256 * b + P * (qh + 1)]
    return full.reshape(B, S, D)


def run(x, Wq, Wk, Wv, Wo, trace=False):
    from concourse.bass_utils import run_bass_kernel_spmd

    nc = _get_nc()
    in_maps = make_in_maps(x, Wq, Wk, Wv, Wo)
    res = run_bass_kernel_spmd(nc, in_maps, list(range(NCORES)), trace=trace)
    return unshard([res.results[r]["out"] for r in range(NCORES)]), res


def kernel(x, Wq, Wk, Wv, Wo):
    out, _ = run(x, Wq, Wk, Wv, Wo)
    if not np.isfinite(out).all():
        # transient device-state flake: retry once
        out, _ = run(x, Wq, Wk, Wv, Wo)
    return out



# revision 27
# speedup vs baseline: 2.1984x; 2.1984x over previous
"""GQA attention (B=2, S=2048, D=2048, 32 q heads / 8 kv heads, RoPE, causal)
sharded over 8 trn2 NeuronCores: tensor-parallel over heads (4 q heads + 1 kv
head per core), AllToAll to redistribute attention output by sequence slice,
each core computing its 512-row slice of the output projection.

v2: bf16 operands everywhere (fp32 PSUM accumulation), KV-packed projection
chain, full causal block-skip with a single shared 128x128 triangular additive
mask, software-pipelined attention (scores / exp / PV with one-unit skew so the
PE never waits on the Act engine), softmax normalization via the fast DVE
reciprocal, batched bf16 AllToAll payloads, and batch-outer output projection
so the last collective hides under the first batch's projection.

Self-contained: hardcodes shapes; builds one SPMD Bass/Tile program and runs it
via run_bass_kernel_spmd on cores 0-7.
"""

import os
import sys
from contextlib import ExitStack

import numpy as np

for _p in ("/opt/trn_rl_repo", "/root/.axon_site/_ro/trn_rl_repo"):
    if os.path.isdir(_p) and _p not in sys.path:
        sys.path.insert(0, _p)

B = 2
S = 2048
D = 2048
HD = 64          # head dim
HQ = 4           # q heads per core
NCORES = 8
ROPE_THETA = 10000.0
P = 128

_CACHE: dict = {}


def _build_program(phases=("xqkv", "attn", "cc", "proj"), reps=1, dbg=False):
    import concourse.bacc as bacc
    import concourse.tile as tile
    from concourse import mybir
    from concourse.masks import make_identity
    import concourse.hw_specs as hw_specs

    # Pin every activation we use (Exp/Copy) to the one table set that has
    # them all, so the table-load pass emits a single LoadActFuncSet.
    if not hasattr(bacc, "_act_tables_pinned"):
        _orig_gat = hw_specs.get_activation_tables
        _AF = mybir.ActivationFunctionType
        _ours = {_AF.Exp, _AF.Ln, _AF.Copy, _AF.Identity}

        def _gat(arch):
            tabs = _orig_gat(arch)
            for name in tabs:
                if name != "natural_log_exp_and_others":
                    tabs[name] = tabs[name] - _ours
            return tabs

        bacc.get_activation_tables = _gat
        bacc._act_tables_pinned = True

    dt = mybir.dt.float32
    BF = mybir.dt.bfloat16
    AF = mybir.ActivationFunctionType
    OP = mybir.AluOpType

    nc = bacc.Bacc("TRN2", target_bir_lowering=False, debug=False, num_devices=NCORES)

    x = nc.dram_tensor("x", [B, S, D], BF, kind="ExternalInput")
    wqT = nc.dram_tensor("wqT", [D, 256], BF, kind="ExternalInput")
    wkvT = nc.dram_tensor("wkvT", [D, 128], BF, kind="ExternalInput")
    woT = nc.dram_tensor("woT", [D, D], BF, kind="ExternalInput")
    ctab = nc.dram_tensor("ctab", [P, S], BF, kind="ExternalInput")
    stab = nc.dram_tensor("stab", [P, S], BF, kind="ExternalInput")
    pswT = nc.dram_tensor("pswT", [P, P], BF, kind="ExternalInput")
    pswkvT = nc.dram_tensor("pswkvT", [HD, HD], BF, kind="ExternalInput")
    maskT = nc.dram_tensor("maskT", [P, P], BF, kind="ExternalInput")
    out = nc.dram_tensor("out", [512, D], dt, kind="ExternalOutput")
    a2a_in = [[nc.dram_tensor(f"a2a_in{i}_{q}", [2048, 128], BF) for q in range(2)] for i in range(B)]
    a2a_out = [[nc.dram_tensor(f"a2a_out{i}_{q}", [2048, 128], BF) for q in range(2)] for i in range(B)]
    if dbg:
        qT_d = nc.dram_tensor("qT_d", [P, 2, S], BF, kind="ExternalOutput")
        k2T_d = nc.dram_tensor("k2T_d", [P, S], BF, kind="ExternalOutput")
        vsb_d = nc.dram_tensor("vsb_d", [P, 16, P], BF, kind="ExternalOutput")
        a2ad = [nc.dram_tensor(f"a2ad_{q}", [2048, 128], BF, kind="ExternalOutput")
                for q in range(2)]

    with tile.TileContext(nc) as tc, ExitStack() as ctx:
        singles = ctx.enter_context(tc.tile_pool(name="singles", bufs=1))
        ident = singles.tile([P, P], dt)
        make_identity(nc, ident)
        identB = singles.tile([P, P], BF)
        nc.vector.tensor_copy(identB, ident)
        psw_sb = singles.tile([P, P], BF)
        nc.sync.dma_start(psw_sb, pswT.ap())
        pswkv_sb = singles.tile([HD, HD], BF)
        nc.sync.dma_start(pswkv_sb, pswkvT.ap())
        msk_sb = singles.tile([P, P], BF)
        nc.sync.dma_start(msk_sb, maskT.ap())
        wq_sb = singles.tile([P, 16, 256], BF)
        wkv_sb = singles.tile([P, 16, 128], BF)
        c_sb = singles.tile([P, S], BF)
        s_sb = singles.tile([P, S], BF)

        # weight loads, chunked and interleaved between the first x-chunk DMAs
        # so neither the first transposes nor the first chain steps wait on a
        # monolithic weight transfer
        prefetch = []
        for c4 in range(4):
            prefetch.append((wq_sb[:, c4 * 4:(c4 + 1) * 4, :],
                             wqT.ap()[c4 * 512:(c4 + 1) * 512].rearrange(
                                 "(ko p) m -> p ko m", p=P)))
        for c4 in range(2):
            prefetch.append((wkv_sb[:, c4 * 8:(c4 + 1) * 8, :],
                             wkvT.ap()[c4 * 1024:(c4 + 1) * 1024].rearrange(
                                 "(ko p) m -> p ko m", p=P)))
        for c4 in range(2):
            prefetch.append((c_sb[:, c4 * 1024:(c4 + 1) * 1024],
                             ctab.ap()[:, c4 * 1024:(c4 + 1) * 1024]))
            prefetch.append((s_sb[:, c4 * 1024:(c4 + 1) * 1024],
                             stab.ap()[:, c4 * 1024:(c4 + 1) * 1024]))
        prefetch = prefetch[::-1]  # pop() from the front

        # SBUF pools live for the whole program; PSUM pools are phase-scoped.
        qTp = ctx.enter_context(tc.tile_pool(name="qTp", bufs=2))
        kvp = ctx.enter_context(tc.tile_pool(name="kvp", bufs=1))
        xtp = ctx.enter_context(tc.tile_pool(name="xtp", bufs=2))
        xnp = ctx.enter_context(tc.tile_pool(name="xnp", bufs=3))
        tmpp = ctx.enter_context(tc.tile_pool(name="tmpp", bufs=2))
        wop = ctx.enter_context(tc.tile_pool(name="wop", bufs=4))
        gp = ctx.enter_context(tc.tile_pool(name="gp", bufs=4))
        obp = ctx.enter_context(tc.tile_pool(name="obp", bufs=2))
        atx = ctx.enter_context(tc.tile_pool(name="atx", bufs=3))
        drp = ctx.enter_context(tc.tile_pool(name="drp", bufs=4))

        for rep in range(reps):
            wos = {}
            g_sbs = {}

            for b in range(B):
                qT = qTp.tile([P, 2, S], BF, tag="qT")
                k2T = kvp.tile([P, S], BF, tag="k2T")
                vsb = kvp.tile([P, 16, P], BF, tag="vsb")
                nc.vector.memset(vsb[:, :, HD:P], 1.0)

                def emit_wo():
                    # wo prefetch on the SWDGE queue. The guard read makes each
                    # wo DMA WAR-depend on b=0's mid-projection state, so the
                    # scheduler cannot hoist the 25us of wo traffic to t=0
                    # where it would starve the startup x loads.
                    for n in range(4):
                        wo_sb = wop.tile([P, 16, 512], BF, tag="wo", name=f"wo{rep}_{n}")
                        guard = drp.tile([1, 1], BF, tag="guard", bufs=1)
                        nc.vector.tensor_tensor(
                            guard, wo_sb[0:1, 0, 0:1], qT[0:1, 0, 1024:1025],
                            OP.mult)
                        nc.gpsimd.dma_start(
                            wo_sb, woT.ap()[:, n * 512:(n + 1) * 512].rearrange(
                                "(ko p) f -> p ko f", p=P))
                        wos[n] = wo_sb

                if b == 1 and "proj" in phases and "attn" not in phases:
                    emit_wo()

                # ---- x^T + QKV projections + RoPE, per 512-col s-block.
                # Chain steps trail the transposes by one 128-row chunk so the
                # PE alternates transpose/chain work with no serial barrier. ----
                with tc.tile_pool(name=f"pst{rep}_{b}", bufs=2, space="PSUM") as pst, \
                     tc.tile_pool(name=f"psq{rep}_{b}", bufs=1, space="PSUM") as psq, \
                     tc.tile_pool(name=f"pswp{rep}_{b}", bufs=1, space="PSUM") as pswp:
                    for n4 in range(4):
                        xTblk = xtp.tile([P, 16, 512], BF, tag="xT")
                        pss3 = [psq.tile([P, 512], dt, tag=f"ps{m}", name=f"ps{m}", bufs=1)
                                for m in range(3)]
                        for sc4 in range(5):
                            if sc4 < 4:
                                sc = n4 * 4 + sc4
                                xa = xnp.tile([P, D], BF, tag="xa")
                                nc.sync.dma_start(xa, x.ap()[b, sc * P:(sc + 1) * P, :])
                                for _ in range(3):
                                    if rep == 0 and b == 0 and prefetch:
                                        dst_w, src_w = prefetch.pop()
                                        nc.sync.dma_start(dst_w, src_w)
                                for dg in range(4):
                                    pt = pst.tile([P, 512], BF, tag="pt", bufs=2)
                                    for j in range(4):
                                        dc = dg * 4 + j
                                        nc.tensor.transpose(
                                            pt[:, j * P:(j + 1) * P],
                                            xa[:, dc * P:(dc + 1) * P],
                                            identB,
                                        )
                                    dest = xTblk[:, dg * 4:dg * 4 + 4, sc4 * P:(sc4 + 1) * P]
                                    nc.scalar.copy(
                                        out=dest,
                                        in_=pt[:, :].rearrange("p (a c) -> p a c", a=4))
                            if sc4 > 0:
                                # chain over the finished 128-col chunk only
                                cc4 = sc4 - 1
                                colr = slice(cc4 * P, (cc4 + 1) * P)
                                for m in range(3):
                                    for k in range(16):
                                        lhsT = wq_sb[:, k, m * P:(m + 1) * P] if m < 2 \
                                            else wkv_sb[:, k, :]
                                        nc.tensor.matmul(
                                            pss3[m][:, colr], lhsT,
                                            xTblk[:, k, colr],
                                            start=(k == 0), stop=(k == 15),
                                        )
                                if sc4 == 4:
                                    # PSUM->bf16 copies start under the last chains
                                    raws = []
                                    for m in range(3):
                                        raw = tmpp.tile([P, 512], BF, tag=f"raw{m}",
                                                        name=f"raw{m}", bufs=1)
                                        nc.vector.tensor_copy(raw, pss3[m])
                                        raws.append(raw)
                        # m=0,1: Q pairs (RoPE on all 128 rows);
                        # m=2: packed [K | V] (RoPE on K rows 0:64, V passthrough)
                        for m in range(3):
                            raw = raws[m]
                            cols = slice(n4 * 512, (n4 + 1) * 512)
                            if m < 2:
                                sw = pswp.tile([P, 512], dt, tag="sw", bufs=1)
                                nc.tensor.matmul(sw, psw_sb, raw, start=True, stop=True)
                                swb = tmpp.tile([P, 512], BF, tag="swb")
                                nc.vector.tensor_copy(swb, sw)
                                u = tmpp.tile([P, 512], BF, tag="u")
                                nc.vector.tensor_tensor(u, swb, s_sb[:, cols], OP.mult)
                                t2 = tmpp.tile([P, 512], BF, tag="t2")
                                nc.vector.tensor_tensor(t2, raw, c_sb[:, cols], OP.mult)
                                nc.vector.tensor_tensor(qT[:, m, cols], u, t2, OP.add)
                            else:
                                sw = pswp.tile([HD, 512], dt, tag="swk", bufs=1)
                                nc.tensor.matmul(sw, pswkv_sb, raw[0:HD, :], start=True, stop=True)
                                swb = tmpp.tile([HD, 512], BF, tag="swbk")
                                nc.vector.tensor_copy(swb, sw)
                                u = tmpp.tile([HD, 512], BF, tag="uk")
                                nc.vector.tensor_tensor(u, swb, s_sb[0:HD, cols], OP.mult)
                                t2 = tmpp.tile([HD, 512], BF, tag="t2k")
                                nc.vector.tensor_tensor(t2, raw[0:HD, :], c_sb[0:HD, cols], OP.mult)
                                kk = tmpp.tile([HD, 512], BF, tag="kk")
                                nc.vector.tensor_tensor(kk, u, t2, OP.add)
                                # duplicate rotated K into both partition halves
                                nc.vector.tensor_copy(k2T[0:HD, cols], kk)
                                nc.vector.tensor_copy(k2T[HD:P, cols], kk)
                                # V^T (raw rows 64:128, no RoPE) -> vsb via PE transpose
                                pvt = pst.tile([P, 256], BF, tag="pvt", bufs=1)
                                for j in range(4):
                                    nc.tensor.transpose(
                                        pvt[:, j * HD:(j + 1) * HD],
                                        raw[HD:P, j * P:(j + 1) * P],
                                        identB[HD:P, HD:P],
                                    )
                                nc.vector.tensor_copy(
                                    vsb[:, n4 * 4:n4 * 4 + 4, 0:HD],
                                    pvt[:, :].rearrange("p (a c) -> p a c", a=4),
                                )

                if dbg and b == 0:
                    nc.sync.dma_start(qT_d.ap(), qT[:, :, :])
                    nc.sync.dma_start(k2T_d.ap(), k2T[:, :])
                    nc.sync.dma_start(vsb_d.ap(), vsb[:, :, :])

                # ---- attention: flat software-pipelined unit list ----
                if "attn" in phases:
                    units = []
                    for qh in range(2):
                        for p in range(2):
                            qc0, qc1 = 2 * qh, 2 * qh + 1
                            for kc in range(4 * qc1 + 4):
                                for qc in (qc0, qc1):
                                    if kc < 4 * (qc + 1):
                                        units.append((qh, p, qc, kc))

                    with tc.tile_pool(name=f"pss{rep}_{b}", bufs=2, space="PSUM") as pss, \
                         tc.tile_pool(name=f"pspv{rep}_{b}", bufs=1, space="PSUM") as pspv:
                        pvd = {}       # (qh,p,qc) -> psum tile [128, 2, 512]
                        pqd = {}       # unit idx -> pq tile
                        ndone = {0: 0, 1: 0}

                        def emit_S(i):
                            qh, p, qc, kc = units[i]
                            diag = kc >= 4 * qc
                            skip = P * (kc - 4 * qc) if diag else 0
                            sq = pss.tile([P, 1024], dt, tag="sq")
                            for h in range(2):
                                hb = h * 512
                                if diag:
                                    # strip [skip, skip+128): scores + additive
                                    # causal mask; rest [skip+128, 512) plain
                                    nc.tensor.matmul(
                                        sq[:, hb + skip:hb + skip + P],
                                        k2T[HD * h:HD * (h + 1), kc * P:(kc + 1) * P],
                                        qT[HD * h:HD * (h + 1), p, qc * 512 + skip:qc * 512 + skip + P],
                                        start=True, stop=False,
                                        tile_position=(HD * h, 0),
                                        skip_group_check=True,
                                    )
                                    nc.tensor.matmul(
                                        sq[:, hb + skip:hb + skip + P],
                                        identB, msk_sb,
                                        start=False, stop=True,
                                        skip_group_check=True,
                                    )
                                    if skip + P < 512:
                                        nc.tensor.matmul(
                                            sq[:, hb + skip + P:hb + 512],
                                            k2T[HD * h:HD * (h + 1), kc * P:(kc + 1) * P],
                                            qT[HD * h:HD * (h + 1), p, qc * 512 + skip + P:(qc + 1) * 512],
                                            start=True, stop=True,
                                            tile_position=(HD * h, 0),
                                            skip_group_check=True,
                                        )
                                else:
                                    nc.tensor.matmul(
                                        sq[:, hb:hb + 512],
                                        k2T[HD * h:HD * (h + 1), kc * P:(kc + 1) * P],
                                        qT[HD * h:HD * (h + 1), p, qc * 512:(qc + 1) * 512],
                                        start=True, stop=True,
                                        tile_position=(HD * h, 0),
                                        skip_group_check=True,
                                    )
                            pq = atx.tile([P, 1024], BF, tag="pq")
                            if skip == 0:
                                nc.scalar.activation(pq, sq, AF.Exp)
                            else:
                                nc.scalar.activation(pq[:, skip:512], sq[:, skip:512], AF.Exp)
                                nc.scalar.activation(
                                    pq[:, 512 + skip:1024], sq[:, 512 + skip:1024], AF.Exp)
                            pqd[i] = pq

                        def emit_drain(key):
                            qh, p, qc = key
                            for h in range(2):
                                t_ = pvd.pop(key + (h,))
                                # custom-DVE ops ignore the input partition base:
                                # copy the denominator rows down to base 0 first
                                den = drp.tile([HD, 512], dt, tag="den", bufs=2)
                                nc.vector.tensor_copy(den, t_[HD:P, :])
                                rc = drp.tile([HD, 512], dt, tag="rc", bufs=2)
                                nc.vector.reciprocal_approx_fast(out=rc, in_=den)
                                at = drp.tile([HD, 512], BF, tag="at", bufs=3)
                                nc.vector.tensor_tensor(at, t_[0:HD, :], rc, OP.mult)
                                base = 1024 * (qc - 2 * qh)
                                off = P * p + HD * h
                                dst = a2a_in[b][qh].ap()[base:base + 1024].rearrange(
                                    "(j s) q -> s j q", j=4)[off:off + HD]
                                nc.sync.dma_start(
                                    dst, at[:, :].rearrange("p (j q) -> p j q", j=4))
                            ndone[qh] += 1
                            if ndone[qh] == 4 and dbg and b == 0:
                                nc.sync.dma_start(a2ad[qh].ap(), a2a_in[b][qh].ap())
                            if ndone[qh] == 4 and "cc" in phases:
                                nc.gpsimd.collective_compute(
                                    "AllToAll",
                                    mybir.AluOpType.bypass,
                                    replica_groups=[list(range(NCORES))],
                                    ins=[a2a_in[b][qh].ap().opt()],
                                    outs=[a2a_out[b][qh].ap().opt()],
                                )
                                if "proj" in phases:
                                    g_sb = gp.tile([P, 16, P], BF, tag="g",
                                                   name=f"g{rep}_{b}{qh}")
                                    nc.gpsimd.dma_start(
                                        g_sb, a2a_out[b][qh].ap().rearrange(
                                            "(ko p) q -> p ko q", p=P))
                                    g_sbs[(b, qh)] = g_sb

                        def emit_P(i):
                            qh, p, qc, kc = units[i]
                            diag = kc >= 4 * qc
                            skip = P * (kc - 4 * qc) if diag else 0
                            pq = pqd.pop(i)
                            key = (qh, p, qc)
                            if kc == 0:
                                for h in range(2):
                                    pvd[key + (h,)] = pspv.tile(
                                        [P, 512], dt, tag=f"pv{qc % 2}{h}", name=f"pv{qc % 2}{h}")
                            last = kc == 4 * (qc + 1) - 1
                            for h in range(2):
                                nc.tensor.matmul(
                                    pvd[key + (h,)][:, skip:], vsb[:, kc, :],
                                    pq[:, h * 512 + skip:(h + 1) * 512],
                                    start=(kc == 0), stop=last,
                                    skip_group_check=True)
                            if last:
                                emit_drain(key)

                        for i in range(len(units)):
                            emit_S(i)
                            if b == 0 and "proj" in phases and i == len(units) // 2:
                                emit_wo()
                            if i > 0:
                                emit_P(i - 1)
                        emit_P(len(units) - 1)

            # ---- output projection: batch-outer so proj(b=0) hides the
            # tail of b=1's collectives ----
            if "proj" in phases:
                with tc.tile_pool(name=f"psop{rep}", bufs=2, space="PSUM") as psop:
                    for bb in range(B):
                        for q2 in range(2):
                            for n in range(4):
                                po = psop.tile([P, 512], dt, tag="po")
                                for k in range(16):
                                    nc.tensor.matmul(
                                        po, g_sbs[(bb, q2)][:, k, :], wos[n][:, k, :],
                                        start=(k == 0), stop=(k == 15))
                                ob = obp.tile([P, 512], dt, tag="ob")
                                nc.vector.tensor_copy(ob, po)
                                nc.sync.dma_start(
                                    out.ap()[256 * bb + P * q2:256 * bb + P * (q2 + 1),
                                             n * 512:(n + 1) * 512], ob)

    nc.compile()
    return nc


def _host_prep(Wq, Wk, Wv, Wo):
    """Per-core weight slices (head-dim permuted, transposed, bf16) + tables."""
    import ml_dtypes
    bf16 = ml_dtypes.bfloat16

    perm = np.concatenate([np.arange(0, HD, 2), np.arange(1, HD, 2)])
    per_core = []
    for r in range(NCORES):
        wq = Wq[256 * r:256 * (r + 1)].reshape(HQ, HD, D)[:, perm].reshape(256, D)
        wqT = np.ascontiguousarray(wq.T * np.float32(0.125)).astype(bf16)
        wk = Wk[HD * r:HD * (r + 1)][perm]
        wv = Wv[HD * r:HD * (r + 1)]
        wkvT = np.ascontiguousarray(np.concatenate([wk, wv], 0).T).astype(bf16)
        per_core.append((wqT, wkvT))
    woT = np.ascontiguousarray(Wo.T).astype(bf16)

    half = HD // 2
    inv = 1.0 / (ROPE_THETA ** (np.arange(half, dtype=np.float64) * 2.0 / HD))
    ang = np.arange(S, dtype=np.float64)[None, :] * inv[:, None]  # [32, S]
    ctab = np.ascontiguousarray(np.tile(np.cos(ang), (4, 1))).astype(bf16)
    stab = np.ascontiguousarray(np.tile(np.sin(ang), (4, 1))).astype(bf16)

    I32 = np.eye(32, dtype=np.float32)
    z = np.zeros((32, 32), np.float32)
    blk = np.block([[z, -I32], [I32, z]])
    pswT = np.ascontiguousarray(np.kron(np.eye(2, dtype=np.float32), blk).T).astype(bf16)
    pswkvT = np.ascontiguousarray(blk.T).astype(bf16)

    rr = np.arange(P)[:, None]
    cc = np.arange(P)[None, :]
    maskT = np.where(cc < rr, np.float32(-1e9), np.float32(0.0)).astype(bf16)
    return per_core, woT, ctab, stab, pswT, pswkvT, maskT


def _get_nc():
    if "nc" not in _CACHE:
        _CACHE["nc"] = _build_program()
    return _CACHE["nc"]


def make_in_maps(x, Wq, Wk, Wv, Wo):
    import ml_dtypes
    bf16 = ml_dtypes.bfloat16

    x = np.ascontiguousarray(np.asarray(x, np.float32)).astype(bf16)
    Wq = np.asarray(Wq, np.float32)
    Wk = np.asarray(Wk, np.float32)
    Wv = np.asarray(Wv, np.float32)
    Wo = np.asarray(Wo, np.float32)
    per_core, woT, ctab, stab, pswT, pswkvT, maskT = _host_prep(Wq, Wk, Wv, Wo)
    in_maps = []
    for r in range(NCORES):
        wqT, wkvT = per_core[r]
        in_maps.append({
            "x": x, "wqT": wqT, "wkvT": wkvT, "woT": woT,
            "ctab": ctab, "stab": stab, "pswT": pswT, "pswkvT": pswkvT,
            "maskT": maskT,
        })
    return in_maps


def unshard(per_core_outs):
    """per_core_outs[r]: [512, D] — blocks of 128 rows = (b, qh) slices."""
    full = np.empty((B * S, D), np.float32)
    for r in range(NCORES):
        o = per_core_outs[r]
        for b in range(B):
            for qh in range(2):
                full[S * b + 1024 * qh + P * r:S * b + 1024 * qh + P * (r + 1)] = \
                    o[256 * b + P * qh:256 * b + P * (qh + 1)]
    return full.reshape(B, S, D)


def run(x, Wq, Wk, Wv, Wo, trace=False):
    from concourse.bass_utils import run_bass_kernel_spmd

    nc = _get_nc()
    in_maps = make_in_maps(x, Wq, Wk, Wv, Wo)
    res = run_bass_kernel_spmd(nc, in_maps, list(range(NCORES)), trace=trace)
    return unshard([res.results[r]["out"] for r in range(NCORES)]), res


def kernel(x, Wq, Wk, Wv, Wo):
    out, _ = run(x, Wq, Wk, Wv, Wo)
    if not np.isfinite(out).all():
        # transient device-state flake: retry once
        out, _ = run(x, Wq, Wk, Wv, Wo)
    return out


# revision 28
# speedup vs baseline: 4.6469x; 2.1137x over previous
"""GQA attention (B=2, S=2048, D=2048, 32 q heads / 8 kv heads, RoPE, causal)
sharded over 8 trn2 NeuronCores: tensor-parallel over heads (4 q heads + 1 kv
head per core), AllToAll to redistribute attention output by sequence slice,
each core computing its 512-row slice of the output projection.

v2: bf16 operands everywhere (fp32 PSUM accumulation), KV-packed projection
chain, full causal block-skip with a single shared 128x128 triangular additive
mask, software-pipelined attention (scores / exp / PV with one-unit skew so the
PE never waits on the Act engine), softmax normalization via the fast DVE
reciprocal, batched bf16 AllToAll payloads, and batch-outer output projection
so the last collective hides under the first batch's projection.

Self-contained: hardcodes shapes; builds one SPMD Bass/Tile program and runs it
via run_bass_kernel_spmd on cores 0-7.
"""

import os
import sys
from contextlib import ExitStack

import numpy as np

for _p in ("/opt/trn_rl_repo", "/root/.axon_site/_ro/trn_rl_repo"):
    if os.path.isdir(_p) and _p not in sys.path:
        sys.path.insert(0, _p)

B = 2
S = 2048
D = 2048
HD = 64          # head dim
HQ = 4           # q heads per core
NCORES = 8
ROPE_THETA = 10000.0
P = 128

_CACHE: dict = {}


def _build_program(phases=("xqkv", "attn", "cc", "proj"), reps=1, dbg=False):
    import concourse.bacc as bacc
    import concourse.tile as tile
    from concourse import mybir
    from concourse.masks import make_identity
    import concourse.hw_specs as hw_specs

    # Pin every activation we use (Exp/Copy) to the one table set that has
    # them all, so the table-load pass emits a single LoadActFuncSet.
    if not hasattr(bacc, "_act_tables_pinned"):
        _orig_gat = hw_specs.get_activation_tables
        _AF = mybir.ActivationFunctionType
        _ours = {_AF.Exp, _AF.Ln, _AF.Copy, _AF.Identity}

        def _gat(arch):
            tabs = _orig_gat(arch)
            for name in tabs:
                if name != "natural_log_exp_and_others":
                    tabs[name] = tabs[name] - _ours
            return tabs

        bacc.get_activation_tables = _gat
        bacc._act_tables_pinned = True

    dt = mybir.dt.float32
    BF = mybir.dt.bfloat16
    AF = mybir.ActivationFunctionType
    OP = mybir.AluOpType

    nc = bacc.Bacc("TRN2", target_bir_lowering=False, debug=False, num_devices=NCORES)

    x = nc.dram_tensor("x", [B, S, D], BF, kind="ExternalInput")
    wqT = nc.dram_tensor("wqT", [D, 256], BF, kind="ExternalInput")
    wkvT = nc.dram_tensor("wkvT", [D, 128], BF, kind="ExternalInput")
    woT = nc.dram_tensor("woT", [D, D], BF, kind="ExternalInput")
    ctab = nc.dram_tensor("ctab", [P, S], BF, kind="ExternalInput")
    stab = nc.dram_tensor("stab", [P, S], BF, kind="ExternalInput")
    pswT = nc.dram_tensor("pswT", [P, P], BF, kind="ExternalInput")
    pswkvT = nc.dram_tensor("pswkvT", [HD, HD], BF, kind="ExternalInput")
    maskT = nc.dram_tensor("maskT", [P, P], BF, kind="ExternalInput")
    out = nc.dram_tensor("out", [512, D], dt, kind="ExternalOutput")
    a2a_in = [[nc.dram_tensor(f"a2a_in{i}_{q}", [2048, 128], BF) for q in range(2)] for i in range(B)]
    a2a_out = [[nc.dram_tensor(f"a2a_out{i}_{q}", [2048, 128], BF) for q in range(2)] for i in range(B)]
    if dbg:
        qT_d = nc.dram_tensor("qT_d", [P, 2, S], BF, kind="ExternalOutput")
        k2T_d = nc.dram_tensor("k2T_d", [P, S], BF, kind="ExternalOutput")
        vsb_d = nc.dram_tensor("vsb_d", [P, 16, P], BF, kind="ExternalOutput")
        a2ad = [nc.dram_tensor(f"a2ad_{q}", [2048, 128], BF, kind="ExternalOutput")
                for q in range(2)]

    with tile.TileContext(nc) as tc, ExitStack() as ctx:
        singles = ctx.enter_context(tc.tile_pool(name="singles", bufs=1))
        ident = singles.tile([P, P], dt)
        make_identity(nc, ident)
        identB = singles.tile([P, P], BF)
        nc.vector.tensor_copy(identB, ident)
        psw_sb = singles.tile([P, P], BF)
        nc.sync.dma_start(psw_sb, pswT.ap())
        pswkv_sb = singles.tile([HD, HD], BF)
        nc.sync.dma_start(pswkv_sb, pswkvT.ap())
        msk_sb = singles.tile([P, P], BF)
        nc.sync.dma_start(msk_sb, maskT.ap())
        wq_sb = singles.tile([P, 16, 256], BF)
        wkv_sb = singles.tile([P, 16, 128], BF)
        c_sb = singles.tile([P, S], BF)
        s_sb = singles.tile([P, S], BF)

        # weight loads, chunked and interleaved between the first x-chunk DMAs
        # so neither the first transposes nor the first chain steps wait on a
        # monolithic weight transfer
        prefetch = []
        for c4 in range(4):
            prefetch.append((wq_sb[:, c4 * 4:(c4 + 1) * 4, :],
                             wqT.ap()[c4 * 512:(c4 + 1) * 512].rearrange(
                                 "(ko p) m -> p ko m", p=P)))
        for c4 in range(2):
            prefetch.append((wkv_sb[:, c4 * 8:(c4 + 1) * 8, :],
                             wkvT.ap()[c4 * 1024:(c4 + 1) * 1024].rearrange(
                                 "(ko p) m -> p ko m", p=P)))
        for c4 in range(2):
            prefetch.append((c_sb[:, c4 * 1024:(c4 + 1) * 1024],
                             ctab.ap()[:, c4 * 1024:(c4 + 1) * 1024]))
            prefetch.append((s_sb[:, c4 * 1024:(c4 + 1) * 1024],
                             stab.ap()[:, c4 * 1024:(c4 + 1) * 1024]))
        prefetch = prefetch[::-1]  # pop() from the front

        # SBUF pools live for the whole program; PSUM pools are phase-scoped.
        qTp = ctx.enter_context(tc.tile_pool(name="qTp", bufs=2))
        kvp = ctx.enter_context(tc.tile_pool(name="kvp", bufs=1))
        xtp = ctx.enter_context(tc.tile_pool(name="xtp", bufs=2))
        xnp = ctx.enter_context(tc.tile_pool(name="xnp", bufs=3))
        tmpp = ctx.enter_context(tc.tile_pool(name="tmpp", bufs=2))
        wop = ctx.enter_context(tc.tile_pool(name="wop", bufs=4))
        gp = ctx.enter_context(tc.tile_pool(name="gp", bufs=4))
        obp = ctx.enter_context(tc.tile_pool(name="obp", bufs=2))
        atx = ctx.enter_context(tc.tile_pool(name="atx", bufs=3))
        drp = ctx.enter_context(tc.tile_pool(name="drp", bufs=4))

        for rep in range(reps):
            wos = {}
            g_sbs = {}

            for b in range(B):
                qT = qTp.tile([P, 2, S], BF, tag="qT")
                k2T = kvp.tile([P, S], BF, tag="k2T")
                vsb = kvp.tile([P, 16, P], BF, tag="vsb")
                nc.vector.memset(vsb[:, :, HD:P], 1.0)

                def emit_wo():
                    # wo prefetch on the SWDGE queue. The guard read makes each
                    # wo DMA WAR-depend on b=0's mid-projection state, so the
                    # scheduler cannot hoist the 25us of wo traffic to t=0
                    # where it would starve the startup x loads.
                    for n in range(4):
                        wo_sb = wop.tile([P, 16, 512], BF, tag="wo", name=f"wo{rep}_{n}")
                        guard = drp.tile([1, 1], BF, tag="guard", bufs=1)
                        nc.vector.tensor_tensor(
                            guard, wo_sb[0:1, 0, 0:1], qT[0:1, 0, 1024:1025],
                            OP.mult)
                        nc.gpsimd.dma_start(
                            wo_sb, woT.ap()[:, n * 512:(n + 1) * 512].rearrange(
                                "(ko p) f -> p ko f", p=P))
                        wos[n] = wo_sb

                if b == 1 and "proj" in phases and "attn" not in phases:
                    emit_wo()

                # ---- x^T + QKV projections + RoPE, per 512-col s-block.
                # Chain steps trail the transposes by one 128-row chunk so the
                # PE alternates transpose/chain work with no serial barrier. ----
                with tc.tile_pool(name=f"pst{rep}_{b}", bufs=2, space="PSUM") as pst, \
                     tc.tile_pool(name=f"psq{rep}_{b}", bufs=1, space="PSUM") as psq, \
                     tc.tile_pool(name=f"pswp{rep}_{b}", bufs=1, space="PSUM") as pswp:
                    for n4 in range(4):
                        xTblk = xtp.tile([P, 16, 512], BF, tag="xT")
                        pss3 = [psq.tile([P, 512], dt, tag=f"ps{m}", name=f"ps{m}", bufs=1)
                                for m in range(3)]
                        for sc4 in range(5):
                            if sc4 < 4:
                                sc = n4 * 4 + sc4
                                xa = xnp.tile([P, D], BF, tag="xa")
                                nc.sync.dma_start(xa, x.ap()[b, sc * P:(sc + 1) * P, :])
                                for _ in range(3):
                                    if rep == 0 and b == 0 and prefetch:
                                        dst_w, src_w = prefetch.pop()
                                        nc.sync.dma_start(dst_w, src_w)
                                for dg in range(4):
                                    pt = pst.tile([P, 512], BF, tag="pt", bufs=2)
                                    for j in range(4):
                                        dc = dg * 4 + j
                                        nc.tensor.transpose(
                                            pt[:, j * P:(j + 1) * P],
                                            xa[:, dc * P:(dc + 1) * P],
                                            identB,
                                        )
                                    dest = xTblk[:, dg * 4:dg * 4 + 4, sc4 * P:(sc4 + 1) * P]
                                    nc.scalar.copy(
                                        out=dest,
                                        in_=pt[:, :].rearrange("p (a c) -> p a c", a=4))
                            if sc4 > 0:
                                # chain over the finished 128-col chunk only
                                cc4 = sc4 - 1
                                colr = slice(cc4 * P, (cc4 + 1) * P)
                                for m in range(3):
                                    for k in range(16):
                                        lhsT = wq_sb[:, k, m * P:(m + 1) * P] if m < 2 \
                                            else wkv_sb[:, k, :]
                                        nc.tensor.matmul(
                                            pss3[m][:, colr], lhsT,
                                            xTblk[:, k, colr],
                                            start=(k == 0), stop=(k == 15),
                                        )
                                if sc4 == 4:
                                    # PSUM->bf16 copies start under the last chains;
                                    # KV first so its rope/vsb tail clears early
                                    raws = [None, None, None]
                                    for m in (2, 0, 1):
                                        raw = tmpp.tile([P, 512], BF, tag=f"raw{m}",
                                                        name=f"raw{m}", bufs=1)
                                        nc.vector.tensor_copy(raw, pss3[m])
                                        raws[m] = raw
                        # m=2 (packed [K | V]) first so k2T/vsb are ready before
                        # the attention phase; then the Q pairs
                        for m in (2, 0, 1):
                            raw = raws[m]
                            cols = slice(n4 * 512, (n4 + 1) * 512)
                            if m < 2:
                                sw = pswp.tile([P, 512], dt, tag="sw", bufs=1)
                                nc.tensor.matmul(sw, psw_sb, raw, start=True, stop=True)
                                swb = tmpp.tile([P, 512], BF, tag="swb")
                                nc.vector.tensor_copy(swb, sw)
                                u = tmpp.tile([P, 512], BF, tag="u")
                                nc.vector.tensor_tensor(u, swb, s_sb[:, cols], OP.mult)
                                t2 = tmpp.tile([P, 512], BF, tag="t2")
                                nc.vector.tensor_tensor(t2, raw, c_sb[:, cols], OP.mult)
                                nc.vector.tensor_tensor(qT[:, m, cols], u, t2, OP.add)
                            else:
                                sw = pswp.tile([HD, 512], dt, tag="swk", bufs=1)
                                nc.tensor.matmul(sw, pswkv_sb, raw[0:HD, :], start=True, stop=True)
                                swb = tmpp.tile([HD, 512], BF, tag="swbk")
                                nc.vector.tensor_copy(swb, sw)
                                u = tmpp.tile([HD, 512], BF, tag="uk")
                                nc.vector.tensor_tensor(u, swb, s_sb[0:HD, cols], OP.mult)
                                t2 = tmpp.tile([HD, 512], BF, tag="t2k")
                                nc.vector.tensor_tensor(t2, raw[0:HD, :], c_sb[0:HD, cols], OP.mult)
                                kk = tmpp.tile([HD, 512], BF, tag="kk")
                                nc.vector.tensor_tensor(kk, u, t2, OP.add)
                                # duplicate rotated K into both partition halves
                                nc.vector.tensor_copy(k2T[0:HD, cols], kk)
                                nc.vector.tensor_copy(k2T[HD:P, cols], kk)
                                # V^T (raw rows 64:128, no RoPE) -> vsb via PE transpose
                                pvt = pst.tile([P, 256], BF, tag="pvt", bufs=1)
                                for j in range(4):
                                    nc.tensor.transpose(
                                        pvt[:, j * HD:(j + 1) * HD],
                                        raw[HD:P, j * P:(j + 1) * P],
                                        identB[HD:P, HD:P],
                                    )
                                nc.vector.tensor_copy(
                                    vsb[:, n4 * 4:n4 * 4 + 4, 0:HD],
                                    pvt[:, :].rearrange("p (a c) -> p a c", a=4),
                                )

                if dbg and b == 0:
                    nc.sync.dma_start(qT_d.ap(), qT[:, :, :])
                    nc.sync.dma_start(k2T_d.ap(), k2T[:, :])
                    nc.sync.dma_start(vsb_d.ap(), vsb[:, :, :])

                # ---- attention: flat software-pipelined unit list ----
                if "attn" in phases:
                    units = []
                    for qh in range(2):
                        for p in range(2):
                            qc0, qc1 = 2 * qh, 2 * qh + 1
                            for kc in range(4 * qc1 + 4):
                                for qc in (qc0, qc1):
                                    if kc < 4 * (qc + 1):
                                        units.append((qh, p, qc, kc))

                    with tc.tile_pool(name=f"pss{rep}_{b}", bufs=2, space="PSUM") as pss, \
                         tc.tile_pool(name=f"pspv{rep}_{b}", bufs=1, space="PSUM") as pspv:
                        pvd = {}       # (qh,p,qc) -> psum tile [128, 2, 512]
                        pqd = {}       # unit idx -> pq tile
                        ndone = {0: 0, 1: 0}

                        def emit_S(i):
                            qh, p, qc, kc = units[i]
                            diag = kc >= 4 * qc
                            skip = P * (kc - 4 * qc) if diag else 0
                            sq = pss.tile([P, 1024], dt, tag="sq")
                            for h in range(2):
                                hb = h * 512
                                if diag:
                                    # strip [skip, skip+128): scores + additive
                                    # causal mask; rest [skip+128, 512) plain
                                    nc.tensor.matmul(
                                        sq[:, hb + skip:hb + skip + P],
                                        k2T[HD * h:HD * (h + 1), kc * P:(kc + 1) * P],
                                        qT[HD * h:HD * (h + 1), p, qc * 512 + skip:qc * 512 + skip + P],
                                        start=True, stop=False,
                                        tile_position=(HD * h, 0),
                                        skip_group_check=True,
                                    )
                                    nc.tensor.matmul(
                                        sq[:, hb + skip:hb + skip + P],
                                        identB, msk_sb,
                                        start=False, stop=True,
                                        skip_group_check=True,
                                    )
                                    if skip + P < 512:
                                        nc.tensor.matmul(
                                            sq[:, hb + skip + P:hb + 512],
                                            k2T[HD * h:HD * (h + 1), kc * P:(kc + 1) * P],
                                            qT[HD * h:HD * (h + 1), p, qc * 512 + skip + P:(qc + 1) * 512],
                                            start=True, stop=True,
                                            tile_position=(HD * h, 0),
                                            skip_group_check=True,
                                        )
                                else:
                                    nc.tensor.matmul(
                                        sq[:, hb:hb + 512],
                                        k2T[HD * h:HD * (h + 1), kc * P:(kc + 1) * P],
                                        qT[HD * h:HD * (h + 1), p, qc * 512:(qc + 1) * 512],
                                        start=True, stop=True,
                                        tile_position=(HD * h, 0),
                                        skip_group_check=True,
                                    )
                            pq = atx.tile([P, 1024], BF, tag="pq")
                            if skip == 0:
                                nc.scalar.activation(pq, sq, AF.Exp)
                            else:
                                nc.scalar.activation(pq[:, skip:512], sq[:, skip:512], AF.Exp)
                                nc.scalar.activation(
                                    pq[:, 512 + skip:1024], sq[:, 512 + skip:1024], AF.Exp)
                            pqd[i] = pq

                        def emit_drain(key):
                            qh, p, qc = key
                            for h in range(2):
                                t_ = pvd.pop(key + (h,))
                                # custom-DVE ops ignore the input partition base:
                                # copy the denominator rows down to base 0 first
                                den = drp.tile([HD, 512], dt, tag="den", bufs=2)
                                nc.vector.tensor_copy(den, t_[HD:P, :])
                                rc = drp.tile([HD, 512], dt, tag="rc", bufs=2)
                                nc.vector.reciprocal_approx_fast(out=rc, in_=den)
                                at = drp.tile([HD, 512], BF, tag="at", bufs=3)
                                nc.vector.tensor_tensor(at, t_[0:HD, :], rc, OP.mult)
                                base = 1024 * (qc - 2 * qh)
                                off = P * p + HD * h
                                dst = a2a_in[b][qh].ap()[base:base + 1024].rearrange(
                                    "(j s) q -> s j q", j=4)[off:off + HD]
                                nc.sync.dma_start(
                                    dst, at[:, :].rearrange("p (j q) -> p j q", j=4))
                            ndone[qh] += 1
                            if ndone[qh] == 4 and dbg and b == 0:
                                nc.sync.dma_start(a2ad[qh].ap(), a2a_in[b][qh].ap())
                            if ndone[qh] == 4 and "cc" in phases:
                                nc.gpsimd.collective_compute(
                                    "AllToAll",
                                    mybir.AluOpType.bypass,
                                    replica_groups=[list(range(NCORES))],
                                    ins=[a2a_in[b][qh].ap().opt()],
                                    outs=[a2a_out[b][qh].ap().opt()],
                                )
                                if "proj" in phases:
                                    g_sb = gp.tile([P, 16, P], BF, tag="g",
                                                   name=f"g{rep}_{b}{qh}")
                                    nc.gpsimd.dma_start(
                                        g_sb, a2a_out[b][qh].ap().rearrange(
                                            "(ko p) q -> p ko q", p=P))
                                    g_sbs[(b, qh)] = g_sb

                        def emit_P(i):
                            qh, p, qc, kc = units[i]
                            diag = kc >= 4 * qc
                            skip = P * (kc - 4 * qc) if diag else 0
                            pq = pqd.pop(i)
                            key = (qh, p, qc)
                            if kc == 0:
                                for h in range(2):
                                    pvd[key + (h,)] = pspv.tile(
                                        [P, 512], dt, tag=f"pv{qc % 2}{h}", name=f"pv{qc % 2}{h}")
                            last = kc == 4 * (qc + 1) - 1
                            for h in range(2):
                                nc.tensor.matmul(
                                    pvd[key + (h,)][:, skip:], vsb[:, kc, :],
                                    pq[:, h * 512 + skip:(h + 1) * 512],
                                    start=(kc == 0), stop=last,
                                    skip_group_check=True)
                            if last:
                                emit_drain(key)

                        for i in range(len(units)):
                            emit_S(i)
                            if b == 0 and "proj" in phases and i == len(units) // 2:
                                emit_wo()
                            if i > 0:
                                emit_P(i - 1)
                        emit_P(len(units) - 1)

            # ---- output projection: batch-outer so proj(b=0) hides the
            # tail of b=1's collectives ----
            if "proj" in phases:
                with tc.tile_pool(name=f"psop{rep}", bufs=2, space="PSUM") as psop:
                    for bb in range(B):
                        for q2 in range(2):
                            for n in range(4):
                                po = psop.tile([P, 512], dt, tag="po")
                                for k in range(16):
                                    nc.tensor.matmul(
                                        po, g_sbs[(bb, q2)][:, k, :], wos[n][:, k, :],
                                        start=(k == 0), stop=(k == 15))
                                ob = obp.tile([P, 512], dt, tag="ob")
                                nc.vector.tensor_copy(ob, po)
                                nc.sync.dma_start(
                                    out.ap()[256 * bb + P * q2:256 * bb + P * (q2 + 1),
                                             n * 512:(n + 1) * 512], ob)

    nc.compile()
    return nc


def _host_prep(Wq, Wk, Wv, Wo):
    """Per-core weight slices (head-dim permuted, transposed, bf16) + tables."""
    import ml_dtypes
    bf16 = ml_dtypes.bfloat16

    perm = np.concatenate([np.arange(0, HD, 2), np.arange(1, HD, 2)])
    per_core = []
    for r in range(NCORES):
        wq = Wq[256 * r:256 * (r + 1)].reshape(HQ, HD, D)[:, perm].reshape(256, D)
        wqT = np.ascontiguousarray(wq.T * np.float32(0.125)).astype(bf16)
        wk = Wk[HD * r:HD * (r + 1)][perm]
        wv = Wv[HD * r:HD * (r + 1)]
        wkvT = np.ascontiguousarray(np.concatenate([wk, wv], 0).T).astype(bf16)
        per_core.append((wqT, wkvT))
    woT = np.ascontiguousarray(Wo.T).astype(bf16)

    half = HD // 2
    inv = 1.0 / (ROPE_THETA ** (np.arange(half, dtype=np.float64) * 2.0 / HD))
    ang = np.arange(S, dtype=np.float64)[None, :] * inv[:, None]  # [32, S]
    ctab = np.ascontiguousarray(np.tile(np.cos(ang), (4, 1))).astype(bf16)
    stab = np.ascontiguousarray(np.tile(np.sin(ang), (4, 1))).astype(bf16)

    I32 = np.eye(32, dtype=np.float32)
    z = np.zeros((32, 32), np.float32)
    blk = np.block([[z, -I32], [I32, z]])
    pswT = np.ascontiguousarray(np.kron(np.eye(2, dtype=np.float32), blk).T).astype(bf16)
    pswkvT = np.ascontiguousarray(blk.T).astype(bf16)

    rr = np.arange(P)[:, None]
    cc = np.arange(P)[None, :]
    maskT = np.where(cc < rr, np.float32(-1e9), np.float32(0.0)).astype(bf16)
    return per_core, woT, ctab, stab, pswT, pswkvT, maskT


def _get_nc():
    if "nc" not in _CACHE:
        _CACHE["nc"] = _build_program()
    return _CACHE["nc"]


def make_in_maps(x, Wq, Wk, Wv, Wo):
    import ml_dtypes
    bf16 = ml_dtypes.bfloat16

    x = np.ascontiguousarray(np.asarray(x, np.float32)).astype(bf16)
    Wq = np.asarray(Wq, np.float32)
    Wk = np.asarray(Wk, np.float32)
    Wv = np.asarray(Wv, np.float32)
    Wo = np.asarray(Wo, np.float32)
    per_core, woT, ctab, stab, pswT, pswkvT, maskT = _host_prep(Wq, Wk, Wv, Wo)
    in_maps = []
    for r in range(NCORES):
        wqT, wkvT = per_core[r]
        in_maps.append({
            "x": x, "wqT": wqT, "wkvT": wkvT, "woT": woT,
            "ctab": ctab, "stab": stab, "pswT": pswT, "pswkvT": pswkvT,
            "maskT": maskT,
        })
    return in_maps


def unshard(per_core_outs):
    """per_core_outs[r]: [512, D] — blocks of 128 rows = (b, qh) slices."""
    full = np.empty((B * S, D), np.float32)
    for r in range(NCORES):
        o = per_core_outs[r]
        for b in range(B):
            for qh in range(2):
                full[S * b + 1024 * qh + P * r:S * b + 1024 * qh + P * (r + 1)] = \
                    o[256 * b + P * qh:256 * b + P * (qh + 1)]
    return full.reshape(B, S, D)


def run(x, Wq, Wk, Wv, Wo, trace=False):
    from concourse.bass_utils import run_bass_kernel_spmd

    nc = _get_nc()
    in_maps = make_in_maps(x, Wq, Wk, Wv, Wo)
    res = run_bass_kernel_spmd(nc, in_maps, list(range(NCORES)), trace=trace)
    return unshard([res.results[r]["out"] for r in range(NCORES)]), res


def kernel(x, Wq, Wk, Wv, Wo):
    out, _ = run(x, Wq, Wk, Wv, Wo)
    if not np.isfinite(out).all():
        # transient device-state flake: retry once
        out, _ = run(x, Wq, Wk, Wv, Wo)
    return out
